# revision 24
# baseline (speedup 1.0000x reference)
"""GraphVLAD (GraphSage message-passing layer) Trainium2 kernel.

Full-input contract: kernel(**inputs) takes the unsharded numpy inputs and
returns the full [512, 32768] float32 output.

Strategy (hardcoded, per spec sharding hint):
  - Data-parallel over nodes: 8 cores x 64 nodes.
  - Host prep: cast matmul operands to bf16, pre-transpose X so the
    contraction dim (feature g) lands on SBUF partitions; weights replicated.
  - Device: neighbor aggregation (DVE adds), two [512,4096]@[4096,2048]
    matmuls per core (PE, fp32 PSUM accum), exact Gelu (ACT), fp32 residual
    add (DVE), store.
"""

import numpy as np
import ml_dtypes

import concourse.bass as bass
import concourse.tile as tile
from concourse import bacc, mybir
from concourse.bass import ts
from concourse.bass_utils import run_bass_kernel_spmd

P = 128
NCORES = 8
N_TOT = 512
D = 32768
G = 4096
H = 2048
NB = 5
SUB = D // G          # 8
N_LOC = N_TOT // NCORES   # 64 nodes per core
ROWS = N_LOC * SUB        # 512 matmul rows per core
KT = G // P               # 32 k-tiles
NCH = H // 512            # 4 column chunks of 512 per weight matrix
MT = ROWS // P            # 4 row tiles
GK = 8                    # k-tiles per aggregation group
NGRP = KT // GK           # 4 aggregation groups
KH = KT // 2              # k-tiles per W half-block

BF = mybir.dt.bfloat16
F32 = mybir.dt.float32

_BUILT = None


def _build():
    nc = bacc.Bacc("TRN2", target_bir_lowering=False, debug=False,
                   num_devices=NCORES)

    # All matmul operands are host-swizzled to partition-major layout so each
    # DMA reads long contiguous runs per partition.
    xsrcT_d = nc.dram_tensor("xsrcT", [P, KT, ROWS], BF,
                             kind="ExternalInput").ap()
    xnbT_d = nc.dram_tensor("xnbT", [NB, P, KT, ROWS], BF,
                            kind="ExternalInput").ap()
    wself_d = nc.dram_tensor("wself", [P, NCH, KT, 512], BF,
                             kind="ExternalInput").ap()
    wnb_d = nc.dram_tensor("wnb", [P, NCH, KT, 512], BF,
                           kind="ExternalInput").ap()
    xres_d = nc.dram_tensor("xres", [2, P, MT, H], F32,
                            kind="ExternalInput").ap()
    out_d = nc.dram_tensor("out", [ROWS, G], F32, kind="ExternalOutput").ap()

    xsrcT_v = xsrcT_d
    xnbT_v = [xnbT_d[i] for i in range(NB)]
    w_v = [wself_d, wnb_d]

    with tile.TileContext(nc) as tc:
        with (
            tc.tile_pool(name="xt", bufs=1) as xt_pool,
            tc.tile_pool(name="wp", bufs=3) as w_pool,
            tc.tile_pool(name="nbt", bufs=2) as nb_pool,
            tc.tile_pool(name="st", bufs=1) as st_pool,
            tc.tile_pool(name="ps", bufs=6, space="PSUM") as ps_pool,
        ):
            xsrcT = xt_pool.tile([P, KT, ROWS], BF, tag="xsrcT")
            for q in range(2):
                nc.sync.dma_start(xsrcT[:, ts(q, KT // 2)],
                                  xsrcT_v[:, ts(q, KT // 2)])
            agg_tiles = [None] * NGRP

            def emit_agg_group(gi):
                ksl = ts(gi, GK)
                at = xt_pool.tile([P, GK, ROWS], BF, tag=f"agg{gi}")
                agg_tiles[gi] = at
                nc.sync.dma_start(at[:], xnbT_v[0][:, ksl])
                for nb_i in range(1, NB):
                    t = nb_pool.tile([P, GK, ROWS], BF, tag="nbtmp")
                    nc.sync.dma_start(t[:], xnbT_v[nb_i][:, ksl])
                    nc.vector.tensor_add(at[:], at[:], t[:])

            # First agg group early so the nb-matmul phase is never starved;
            # the rest are interleaved between self-phase column chunks.
            emit_agg_group(0)
            agg_emitted = 1

            def lhs_tile(mat, kt, m):
                if mat == 0:
                    return xsrcT[:, kt, ts(m, P)]
                return agg_tiles[kt // GK][:, kt % GK, ts(m, P)]

            for mat in range(2):
                # Residual for this mat half, resident: [P, MT, H] f32.
                xres_sb = st_pool.tile([P, MT, H], F32, tag="xres_sb")
                nc.scalar.dma_start(xres_sb[:], xres_d[mat])
                # Output staging for this mat half: [P, MT, H] f32.
                out_st = st_pool.tile([P, MT, H], F32, tag="out_st")
                for nchunk in range(NCH):
                    whalf = []
                    for q in range(2):
                        wh = w_pool.tile([P, KH, 512], BF, tag="w",
                                         name=f"wh_{mat}_{nchunk}_{q}")
                        whalf.append(wh)
                        nc.sync.dma_start(
                            wh[:], w_v[mat][:, nchunk, ts(q, KH)])
                    for m in range(MT):
                        pt = ps_pool.tile([P, 512], F32, tag="psum")
                        for kt in range(KT):
                            nc.tensor.matmul(
                                pt[:],
                                lhsT=lhs_tile(mat, kt, m),
                                rhs=whalf[kt // KH][:, kt % KH, :],
                                start=(kt == 0),
                                stop=(kt == KT - 1),
                            )
                        gsl = out_st[:, m, ts(nchunk, 512)]
                        nc.scalar.activation(
                            gsl, pt[:], mybir.ActivationFunctionType.Gelu)
                        nc.vector.tensor_add(
                            gsl, gsl, xres_sb[:, m, ts(nchunk, 512)])
                    if mat == 0 and agg_emitted < NGRP:
                        emit_agg_group(agg_emitted)
                        agg_emitted += 1
                for m in range(MT):
                    nc.scalar.dma_start(
                        out_d[ts(m, P), mat * H:(mat + 1) * H],
                        out_st[:, m, :])
    nc.compile()
    return nc


def _swz_T(rows_bf):
    # [ROWS, G] -> [P, KT, ROWS]  (partition-major, contiguous per partition)
    return np.ascontiguousarray(
        rows_bf.reshape(ROWS, KT, P).transpose(2, 1, 0))


def _swz_w(w_bf):
    # [G, H] -> [P, NCH, KT, 512]
    return np.ascontiguousarray(
        w_bf.reshape(KT, P, NCH, 512).transpose(1, 2, 0, 3))


def _prep_core(x_src_c, x_nb_c, w_self_sw, w_nb_sw):
    rows = np.ascontiguousarray(x_src_c.reshape(ROWS, G))
    xsrcT = _swz_T(rows.astype(ml_dtypes.bfloat16))
    nb_rows = (x_nb_c.reshape(N_LOC, NB, SUB, G)
               .transpose(1, 0, 2, 3)
               .reshape(NB, ROWS, G)
               .astype(ml_dtypes.bfloat16))
    xnbT = np.ascontiguousarray(
        nb_rows.reshape(NB, ROWS, KT, P).transpose(0, 3, 2, 1))
    xres_sw = np.ascontiguousarray(
        rows.reshape(MT, P, 2, H).transpose(2, 1, 0, 3))
    return {
        "xsrcT": xsrcT,
        "xnbT": xnbT,
        "wself": w_self_sw,
        "wnb": w_nb_sw,
        "xres": xres_sw,
    }


def kernel(x_src, x_nb, w_self, w_nb, _trace=False):
    global _BUILT
    x_src = np.asarray(x_src, dtype=np.float32)
    x_nb = np.asarray(x_nb, dtype=np.float32)
    w_self_sw = _swz_w(
        np.asarray(w_self, dtype=np.float32).astype(ml_dtypes.bfloat16))
    w_nb_sw = _swz_w(
        np.asarray(w_nb, dtype=np.float32).astype(ml_dtypes.bfloat16))

    if _BUILT is None:
        _BUILT = _build()
    nc = _BUILT

    in_maps = []
    for c in range(NCORES):
        in_maps.append(_prep_core(
            x_src[c * N_LOC:(c + 1) * N_LOC],
            x_nb[c * N_LOC * NB:(c + 1) * N_LOC * NB],
            w_self_sw, w_nb_sw))

    res = run_bass_kernel_spmd(nc, in_maps, list(range(NCORES)),
                               trace=_trace)
    out = np.empty((N_TOT, D), dtype=np.float32)
    for c in range(NCORES):
        out[c * N_LOC:(c + 1) * N_LOC] = res.results[c]["out"].reshape(N_LOC, D)
    if _trace:
        return out, res
    return out
